# revision 25
# baseline (speedup 1.0000x reference)
"""Trainium2 Bass kernel for nn_ARQS (MLP-conditioned rational-quadratic spline).

Contract: kernel(**inputs) takes FULL unsharded inputs (B=65536), shards the
batch across 8 NeuronCores (pure data parallel, weights replicated), runs a
Bass/Tile kernel per core, and returns (z [B,64] f32, logdet_sum [B] f32).

Structure per core (8192 rows): 16 super-tiles x (4 tiles x 128 examples).
  MLP per 128-tile (batch-major, PE):
    x -> PE-transpose -> h0 = relu(x@W0aug) -> transpose -> h1 -> transpose
    -> params in PSUM (W2 columns pre-permuted into widths|heights|derivs).
  Spline phase batched over the 4 tiles of a super-tile (cuts per-op overhead):
    exp/softplus on ACT -> group sums / cumsum / searchsorted masks ->
    telescoped gather dot-products -> RQS algebra, spread across DVE/ACT/Pool.
All scale constants (0.992, 1/8 pool-avg, 1/7 pool-dots) are folded into
scalar immediates of fused scalar_tensor_tensor ops or host-side tables.
"""

import math
import os
import sys

import numpy as np

_TRN_REPO = "/opt/trn_rl_repo"
if _TRN_REPO not in sys.path:
    sys.path.insert(0, _TRN_REPO)

# ---------------------------------------------------------------- constants
B_FULL = 65536
D = 64
H = 512
NB = 8
N_CORES = 8
B_LOC = B_FULL // N_CORES          # 8192
P = 128                            # examples per tile (partition dim)
S = 4                              # tiles per super-tile (spline batch)
OUT_DIM = D * (3 * NB - 1)         # 1472
MIN_BIN = 1e-3
MIN_DERIV = 1e-3
EPS = 1e-6
SCALE = 1.0 - MIN_BIN * NB         # 0.992


def _mm_dtype(mybir, use_bf16):
    return mybir.dt.bfloat16 if use_bf16 else mybir.dt.float32


def build_program(B_loc=B_LOC, use_bf16=True):
    """Build the per-core Bass program. All cores run the same program (SPMD)."""
    import concourse.bass as bass
    import concourse.bacc as bacc
    from concourse import mybir
    from concourse.tile import TileContext

    f32 = mybir.dt.float32
    mdt = _mm_dtype(mybir, use_bf16)
    T = B_loc // P
    NS = T // S                     # super-tiles
    assert T % S == 0
    AF = mybir.ActivationFunctionType
    OP = mybir.AluOpType

    # Force the act-table chooser onto the one set that holds every function
    # we use (exp, ln, relu, square, copy, identity). The greedy chooser
    # otherwise ping-pongs exp_and_others <-> natural_log every tile
    # (~2.7us/tile of table loads). Indices must be preserved (ABI with
    # walrus), so non-chosen sets are emptied rather than removed.
    import concourse.hw_specs as hw_specs
    _orig_tables = hw_specs.get_activation_tables

    def _only_combined(arch):
        tabs = _orig_tables(arch)
        return {
            name: (funcs if name == "natural_log_exp_and_others" else set())
            for name, funcs in tabs.items()
        }

    bacc.get_activation_tables = _only_combined

    nc = bacc.Bacc("TRN2", target_bir_lowering=False, debug=False)

    # ---- DRAM I/O
    x_d = nc.dram_tensor("x", [B_loc, D], f32, kind="ExternalInput").ap()
    w0a_d = nc.dram_tensor("W0a", [D + 1, H], mdt, kind="ExternalInput").ap()
    w1_d = nc.dram_tensor("W1c", [H, H], mdt, kind="ExternalInput").ap()
    b1_d = nc.dram_tensor("b1r", [1, H], mdt, kind="ExternalInput").ap()
    w2_d = nc.dram_tensor("W2P", [H, OUT_DIM], mdt, kind="ExternalInput").ap()
    b2_d = nc.dram_tensor("b2r", [1, OUT_DIM], mdt, kind="ExternalInput").ap()
    idt_d = nc.dram_tensor("IDT", [P, P], mdt, kind="ExternalInput").ap()
    offs_d = nc.dram_tensor("OFFS", [P, NB - 1], f32, kind="ExternalInput").ap()
    z_d = nc.dram_tensor("z", [B_loc, D], f32, kind="ExternalOutput").ap()
    ld_d = nc.dram_tensor("ld", [T, P], f32, kind="ExternalOutput").ap()

    # super-tile views: [NS, P, S, D] with example b = (st*S + s)*P + p
    x_s = x_d.rearrange("(n s p) d -> n p s d", s=S, p=P)
    z_s = z_d.rearrange("(n s p) d -> n p s d", s=S, p=P)

    def bc(ap, dims):
        return bass.AP(tensor=ap.tensor, offset=ap.offset, ap=dims)

    with TileContext(nc) as tc:
        with (
            tc.tile_pool(name="consts", bufs=1) as consts,
            tc.tile_pool(name="mlp", bufs=2) as mlp,
            tc.tile_pool(name="wideP", bufs=2) as wideP,
            tc.tile_pool(name="wtmp", bufs=2) as wtmp,
            tc.tile_pool(name="wt1", bufs=1) as wt1,
            tc.tile_pool(name="small", bufs=1) as small,
            tc.tile_pool(name="alg", bufs=16) as alg,
            tc.tile_pool(name="psT", bufs=2, space="PSUM") as psT,
            tc.tile_pool(name="psM", bufs=2, space="PSUM") as psM,
            tc.tile_pool(name="psO", bufs=4, space="PSUM") as psO,
        ):
            # ---- constants
            w0a = consts.tile([D + 1, H], mdt)
            nc.sync.dma_start(out=w0a, in_=w0a_d)
            w1 = consts.tile([P, 4, H], mdt)
            nc.sync.dma_start(out=w1, in_=w1_d.rearrange("(k p) h -> p k h", p=P))
            b1r = consts.tile([1, H], mdt)
            nc.sync.dma_start(out=b1r, in_=b1_d)
            w2 = consts.tile([P, 4, OUT_DIM], mdt)
            nc.sync.dma_start(out=w2, in_=w2_d.rearrange("(k p) h -> p k h", p=P))
            b2r = consts.tile([1, OUT_DIM], mdt)
            nc.sync.dma_start(out=b2r, in_=b2_d)
            idt = consts.tile([P, P], mdt)
            nc.sync.dma_start(out=idt, in_=idt_d)
            offs = consts.tile([P, NB - 1], f32)   # (8/SCALE)*(i*1e-3), i=1..7
            nc.sync.dma_start(out=offs, in_=offs_d)
            ones1 = consts.tile([1, P], mdt)
            nc.vector.memset(ones1, 1.0)
            epsb = consts.tile([P, 1], f32)
            nc.vector.memset(epsb, EPS)
            ldacc = consts.tile([P, T], f32)

            for st in range(NS):
                # batched spline buffers for this super-tile
                xb = wideP.tile([P, S, D], f32, tag="xb")
                nc.sync.dma_start(out=xb, in_=x_s[st])
                ew = wideP.tile([P, S, D, NB], f32, tag="ew")
                eh = wideP.tile([P, S, D, NB], f32, tag="eh")
                spp = wideP.tile([P, S, D, NB + 1], f32, tag="spp")
                nc.gpsimd.memset(spp[:, :, :, 0:1].rearrange('p s d l -> p (s d) l'), 1.0 - MIN_DERIV)
                nc.gpsimd.memset(spp[:, :, :, NB : NB + 1].rearrange('p s d l -> p (s d) l'), 1.0 - MIN_DERIV)

                for s in range(S):
                    # ========================= MLP (per tile) =========================
                    if use_bf16:
                        xmm = mlp.tile([P, D], mdt, tag="xmm")
                        nc.vector.tensor_copy(xmm, xb[:, s, :])
                    else:
                        xmm = xb[:, s, :]
                    pxT = psT.tile([P, H], mdt, tag="tr")
                    nc.tensor.transpose(pxT[0:D, 0:P], xmm, idt)
                    xTa = mlp.tile([D + 1, P], mdt, tag="xTa")
                    nc.scalar.copy(xTa[0:D, :], pxT[0:D, 0:P])
                    nc.vector.memset(xTa[D : D + 1, :], 1.0)

                    ph0 = psM.tile([P, H], f32, tag="mm")
                    nc.tensor.matmul(ph0, lhsT=xTa, rhs=w0a, start=True, stop=True)
                    h0 = mlp.tile([P, H], mdt, tag="h0")
                    nc.scalar.activation(h0, ph0, AF.Relu)

                    ptr0 = psT.tile([P, H], mdt, tag="tr")
                    for k in range(4):
                        nc.tensor.transpose(ptr0[:, k * P : (k + 1) * P],
                                            h0[:, k * P : (k + 1) * P], idt)
                    h0T = mlp.tile([P, 4, P], mdt, tag="h0T")
                    nc.scalar.copy(h0T[:].rearrange("p k q -> p (k q)"), ptr0)

                    ph1 = psM.tile([P, H], f32, tag="mm")
                    for k in range(4):
                        nc.tensor.matmul(ph1, lhsT=h0T[:, k, :], rhs=w1[:, k, :],
                                         start=(k == 0), stop=False)
                    nc.tensor.matmul(ph1, lhsT=ones1, rhs=b1r, start=False, stop=True)
                    h1 = mlp.tile([P, H], mdt, tag="h1")
                    nc.scalar.activation(h1, ph1, AF.Relu)

                    ptr1 = psT.tile([P, H], mdt, tag="tr")
                    for k in range(4):
                        nc.tensor.transpose(ptr1[:, k * P : (k + 1) * P],
                                            h1[:, k * P : (k + 1) * P], idt)
                    h1T = mlp.tile([P, 4, P], mdt, tag="h1T")
                    nc.scalar.copy(h1T[:].rearrange("p k q -> p (k q)"), ptr1)

                    pw = psO.tile([P, H], f32, tag="out")
                    ph = psO.tile([P, H], f32, tag="out")
                    pd = psO.tile([P, 448], f32, tag="out")
                    for k in range(4):
                        nc.tensor.matmul(pw, lhsT=h1T[:, k, :], rhs=w2[:, k, 0:512],
                                         start=(k == 0), stop=False)
                    nc.tensor.matmul(pw, lhsT=ones1, rhs=b2r[:, 0:512],
                                     start=False, stop=True)
                    for k in range(4):
                        nc.tensor.matmul(ph, lhsT=h1T[:, k, :], rhs=w2[:, k, 512:1024],
                                         start=(k == 0), stop=False)
                    nc.tensor.matmul(ph, lhsT=ones1, rhs=b2r[:, 512:1024],
                                     start=False, stop=True)
                    for k in range(4):
                        nc.tensor.matmul(pd, lhsT=h1T[:, k, :], rhs=w2[:, k, 1024:1472],
                                         start=(k == 0), stop=False)
                    nc.tensor.matmul(pd, lhsT=ones1, rhs=b2r[:, 1024:1472],
                                     start=False, stop=True)

                    # nonlinearities straight out of PSUM into batched buffers
                    nc.scalar.activation(
                        ew[:, s, :, :].rearrange("p g l -> p (g l)"), pw, AF.Exp)
                    nc.scalar.activation(
                        eh[:, s, :, :].rearrange("p g l -> p (g l)"), ph, AF.Exp)
                    spe = mlp.tile([P, 448], f32, tag="spe")
                    nc.scalar.activation(spe, pd, AF.Exp)
                    nc.scalar.activation(
                        spp[:, s, :, 1:NB],
                        spe[:].rearrange("p (g l) -> p g l", l=NB - 1),
                        AF.Ln, bias=1.0)

                # =================== batched spline (super-tile) ===================
                # inclusive cumsum of ew lanes 0..6 (Hillis-Steele, copies on ACT)
                # --- all spline ops run on <=3D flattened views (ISA limit) ---
                def F2(v):   # [P, S, D](-like view) -> [P, S*D]
                    ap = v[:]
                    return ap.rearrange("p s d -> p (s d)") if len(ap.shape) == 3 else ap

                def F3(v):   # [P, S, D, L] view -> [P, S*D, L]
                    ap = v if isinstance(v, bass.AP) else v[:]
                    return ap.rearrange("p s d l -> p (s d) l") if len(ap.shape) == 4 else ap

                c1 = wtmp.tile([P, S, D, NB - 1], f32, tag="hs")
                nc.scalar.copy(F3(c1[:, :, :, 0:1]), F3(ew[:, :, :, 0:1]))
                nc.vector.tensor_add(F3(c1[:, :, :, 1:7]), F3(ew[:, :, :, 1:7]),
                                     F3(ew[:, :, :, 0:6]))
                c2 = wtmp.tile([P, S, D, NB - 1], f32, tag="hs")
                nc.scalar.copy(F3(c2[:, :, :, 0:2]), F3(c1[:, :, :, 0:2]))
                nc.vector.tensor_add(F3(c2[:, :, :, 2:7]), F3(c1[:, :, :, 2:7]),
                                     F3(c1[:, :, :, 0:5]))
                cw = wt1.tile([P, S, D, NB - 1], f32, tag="cw")
                nc.gpsimd.tensor_copy(F3(cw[:, :, :, 0:4]), F3(c2[:, :, :, 0:4]))
                nc.vector.tensor_add(F3(cw[:, :, :, 4:7]), F3(c2[:, :, :, 4:7]),
                                     F3(c2[:, :, :, 0:3]))

                # group sums: sum_w = cum_6 + E_7 (reuses cumsum); sum_h by reduce
                sw = small.tile([P, S, D], f32, tag="sw")
                nc.vector.tensor_add(F2(sw), F2(cw[:, :, :, 6]), F2(ew[:, :, :, 7]))
                sh = small.tile([P, S, D], f32, tag="sh")
                nc.vector.reduce_sum(F2(sh), F3(eh[:]), axis=mybir.AxisListType.X)
                rw = small.tile([P, S, D], f32, tag="rw")   # 1 / sum_w
                nc.vector.reciprocal(F2(rw), F2(sw))
                rh = small.tile([P, S, D], f32, tag="rh")   # 1 / sum_h
                nc.vector.reciprocal(F2(rh), F2(sh))

                # masks g_i = [cum_{i-1} < (x - i*1e-3)/SCALE * sum_w]
                xof = wtmp.tile([P, S, D, NB - 1], f32, tag="cmp")
                xb_f = F2(xb)
                xb_b = bc(xb_f, list(xb_f.ap) + [[0, NB - 1]])
                offs_b = bc(offs[:], [offs[:].ap[0], [0, S * D], offs[:].ap[1]])
                nc.vector.scalar_tensor_tensor(
                    F3(xof[:]), in0=xb_b, scalar=1.0 / SCALE, in1=offs_b,
                    op0=OP.mult, op1=OP.subtract)
                xs = wtmp.tile([P, S, D, NB - 1], f32, tag="cmp")
                sw_f = F2(sw)
                sw_b = bc(sw_f, list(sw_f.ap) + [[0, NB - 1]])
                nc.vector.tensor_tensor(F3(xs[:]), F3(xof[:]), sw_b, op=OP.mult)
                g = wt1.tile([P, S, D, NB - 1], f32, tag="g")
                nc.vector.tensor_tensor(F3(g[:]), F3(cw[:]), F3(xs[:]), op=OP.is_lt)
                kf = small.tile([P, S, D], f32, tag="kf")   # bin index k
                nc.vector.reduce_sum(F2(kf), F3(g[:]), axis=mybir.AxisListType.X)

                # telescoped gather dot-products
                def prod(v, name, eng):
                    pr = wtmp.tile([P, S, D, NB - 1], f32, tag="pr")
                    eng.tensor_mul(F3(pr[:]), F3(g[:]), F3(v))
                    return pr

                def dsum(pr, name):
                    o = small.tile([P, S, D], f32, tag=f"d{name}")
                    nc.vector.reduce_sum(F2(o), F3(pr[:]), axis=mybir.AxisListType.X)
                    return o

                dD = wt1.tile([P, S, D, NB], f32, tag="dD")
                nc.gpsimd.tensor_sub(F3(dD[:]), F3(spp[:, :, :, 1:9]),
                                     F3(spp[:, :, :, 0:8]))

                dA1 = dsum(prod(ew[:, :, :, 0:7], "A1", nc.vector), "A1")
                dA2 = dsum(prod(ew[:, :, :, 1:8], "A2", nc.gpsimd), "A2")
                dC1 = dsum(prod(eh[:, :, :, 0:7], "C1", nc.vector), "C1")
                dC2 = dsum(prod(eh[:, :, :, 1:8], "C2", nc.gpsimd), "C2")
                dK = dsum(prod(dD[:, :, :, 0:7], "K", nc.vector), "K")
                dK1 = dsum(prod(dD[:, :, :, 1:8], "K1", nc.gpsimd), "K1")

                # ---------------- RQS algebra on [P, S*D] ----------------
                def tt(op, a, b_, name, eng=nc.vector):
                    o = alg.tile([P, S, D], f32, tag="alg")
                    eng.tensor_tensor(F2(o), F2(a), F2(b_), op=op)
                    return o

                def stt(in0, scalar, op0, op1, in1, name, eng=nc.vector):
                    o = alg.tile([P, S, D], f32, tag="alg")
                    eng.scalar_tensor_tensor(F2(o), in0=F2(in0), scalar=scalar,
                                             in1=F2(in1), op0=op0, op1=op1)
                    return o

                def ts2(in0, s1, op0, s2, op1, name, eng=nc.vector):
                    o = alg.tile([P, S, D], f32, tag="alg")
                    if s2 is None:
                        eng.tensor_scalar(F2(o), F2(in0), s1, None, op0)
                    else:
                        eng.tensor_scalar(F2(o), F2(in0), s1, s2, op0, op1)
                    return o

                xk1 = stt(dA1, SCALE, OP.mult, OP.mult, rw, "xk1")
                x_k = stt(kf, MIN_BIN, OP.mult, OP.add, xk1, "xk")
                yk1 = stt(dC1, SCALE, OP.mult, OP.mult, rh, "yk1")
                y_k = stt(kf, MIN_BIN, OP.mult, OP.add, yk1, "yk")
                eg1 = tt(OP.subtract, dA2, dA1, "eg1", nc.gpsimd)
                eg2 = tt(OP.add, eg1, ew[:, :, :, 0], "eg2")
                eg3 = stt(eg2, SCALE, OP.mult, OP.mult, rw, "eg3")
                w_k = ts2(eg3, MIN_BIN, OP.add, None, OP.add, "wk")
                hg1 = tt(OP.subtract, dC2, dC1, "hg1", nc.gpsimd)
                hg2 = tt(OP.add, hg1, eh[:, :, :, 0], "hg2", nc.gpsimd)
                hg3 = stt(hg2, SCALE, OP.mult, OP.mult, rh, "hg3")
                h_k = ts2(hg3, MIN_BIN, OP.add, None, OP.add, "hk")
                d_k = ts2(dK, 1.0, OP.add, None, OP.add, "dk")
                d_k1 = stt(dK1, MIN_DERIV, OP.add, OP.add, spp[:, :, :, 1], "dk1")

                rwk = alg.tile([P, S, D], f32, tag="alg")
                nc.vector.reciprocal(F2(rwk), F2(w_k))
                sk = tt(OP.mult, h_k, rwk, "s")
                xmx = tt(OP.subtract, xb, x_k, "xmx", nc.gpsimd)
                th = tt(OP.mult, xmx, rwk, "th")
                th2 = alg.tile([P, S, D], f32, tag="alg")
                nc.scalar.square(F2(th2), F2(th))
                omt = ts2(th, -1.0, OP.mult, 1.0, OP.add, "omt")
                omt2 = alg.tile([P, S, D], f32, tag="alg")
                nc.scalar.square(F2(omt2), F2(omt))
                tq = tt(OP.mult, th, omt, "tq")
                dsum_ = tt(OP.add, d_k, d_k1, "dsum", nc.gpsimd)
                q = stt(sk, -2.0, OP.mult, OP.add, dsum_, "q")
                qt = tt(OP.mult, q, tq, "qt")
                den = tt(OP.add, qt, sk, "den")
                dene = ts2(den, EPS, OP.add, None, OP.add, "dene")
                rden = alg.tile([P, S, D], f32, tag="alg")
                nc.vector.reciprocal(F2(rden), F2(dene))
                sth2 = tt(OP.mult, sk, th2, "sth2", nc.gpsimd)
                dkt = tt(OP.mult, d_k, tq, "dkt", nc.gpsimd)
                num = tt(OP.add, sth2, dkt, "num")
                frac = tt(OP.mult, num, rden, "frac")
                hfrac = tt(OP.mult, h_k, frac, "hfrac")
                zt = wt1.tile([P, S, D], f32, tag="zt")
                nc.vector.tensor_add(F2(zt), F2(y_k), F2(hfrac))
                nc.sync.dma_start(out=z_s[st], in_=zt)

                # log-det
                a1 = tt(OP.mult, d_k1, th2, "a1", nc.gpsimd)
                b1_ = stt(sk, 2.0, OP.mult, OP.mult, tq, "b1")
                c1_ = tt(OP.mult, d_k, omt2, "c1", nc.gpsimd)
                ab = tt(OP.add, a1, b1_, "ab", nc.gpsimd)
                abc = tt(OP.add, ab, c1_, "abc")
                s2t = alg.tile([P, S, D], f32, tag="alg")
                nc.scalar.square(F2(s2t), F2(sk))
                nom = tt(OP.mult, abc, s2t, "nom")
                den2 = alg.tile([P, S, D], f32, tag="alg")
                nc.scalar.square(F2(den2), F2(den))
                lden = alg.tile([P, S, D], f32, tag="alg")
                nc.scalar.activation(F2(lden), F2(den2), AF.Ln, bias=epsb[:])
                lnom = alg.tile([P, S, D], f32, tag="alg")
                nc.scalar.activation(F2(lnom), F2(nom), AF.Ln)
                ldd = tt(OP.subtract, lnom, lden, "ldd")
                nc.vector.reduce_sum(ldacc[:, st * S : (st + 1) * S], ldd,
                                     axis=mybir.AxisListType.X)

            nc.sync.dma_start(out=ld_d.rearrange("t p -> p t"), in_=ldacc)

    nc.finalize()
    return nc


def prep_consts(W0, b0, W1, b1, W2, b2, use_bf16=True):
    """Host-side weight preparation (permutes W2 columns into w|h|d sections)."""
    dt = np.float32
    W0a = np.concatenate([W0, b0[None, :]], axis=0).astype(dt)      # [65, 512]
    newcol = np.empty(OUT_DIM, dtype=np.int64)
    for d in range(D):
        for tn in range(3 * NB - 1):
            old = d * (3 * NB - 1) + tn
            if tn < NB:
                new = d * NB + tn
            elif tn < 2 * NB:
                new = 512 + d * NB + (tn - NB)
            else:
                new = 1024 + d * (NB - 1) + (tn - 2 * NB)
            newcol[old] = new
    W2P = np.empty_like(W2, dtype=dt)
    W2P[:, newcol] = W2
    b2P = np.empty_like(b2, dtype=dt)
    b2P[newcol] = b2
    IDT = np.eye(P, dtype=dt)
    OFFS = np.tile((np.arange(1, NB, dtype=np.float64) * MIN_BIN / SCALE
                    ).astype(dt)[None, :], (P, 1))
    consts = {
        "W0a": W0a,
        "W1c": W1.astype(dt),
        "b1r": b1[None, :].astype(dt),
        "W2P": W2P,
        "b2r": b2P[None, :],
        "IDT": IDT,
        "OFFS": OFFS,
    }
    if use_bf16:
        import ml_dtypes
        for name in ("W0a", "W1c", "b1r", "W2P", "b2r", "IDT"):
            consts[name] = consts[name].astype(ml_dtypes.bfloat16)
    return consts


_PROGRAM_CACHE = {}


def _get_program(use_bf16):
    key = use_bf16
    if key not in _PROGRAM_CACHE:
        _PROGRAM_CACHE[key] = build_program(B_LOC, use_bf16)
    return _PROGRAM_CACHE[key]


USE_BF16 = True


def _run(inputs, trace=False):
    from concourse.bass_utils import run_bass_kernel_spmd

    x = np.asarray(inputs["x"], np.float32)
    consts = prep_consts(
        np.asarray(inputs["W0"], np.float32), np.asarray(inputs["b0"], np.float32),
        np.asarray(inputs["W1"], np.float32), np.asarray(inputs["b1"], np.float32),
        np.asarray(inputs["W2"], np.float32), np.asarray(inputs["b2"], np.float32),
        use_bf16=USE_BF16)
    nc = _get_program(USE_BF16)
    xs = x.reshape(N_CORES, B_LOC, D)
    in_maps = [dict(consts, x=xs[c]) for c in range(N_CORES)]
    res = run_bass_kernel_spmd(nc, in_maps, list(range(N_CORES)), trace=trace)
    z = np.concatenate([r["z"] for r in res.results], axis=0)
    ld = np.concatenate([r["ld"].reshape(-1) for r in res.results], axis=0)
    return (z, ld), res


def kernel(x, W0, b0, W1, b1, W2, b2, num_bins):
    assert int(num_bins) == NB
    (z, ld), _ = _run(dict(x=x, W0=W0, b0=b0, W1=W1, b1=b1, W2=W2, b2=b2))
    return z, ld


def bench(inputs, iters=12):
    """Device-resident repeated execution; returns per-iteration seconds via
    the slope between a short and a long run (subtracts dispatch overhead)."""
    import time
    import jax
    from jax.sharding import Mesh, PartitionSpec
    from jax.experimental.shard_map import shard_map
    from concourse import bass2jax
    from concourse.bass2jax import _bass_exec_p, partition_id_tensor
    import concourse.mybir as mybir

    bass2jax.install_neuronx_cc_hook()
    x = np.asarray(inputs["x"], np.float32)
    consts = prep_consts(
        np.asarray(inputs["W0"], np.float32), np.asarray(inputs["b0"], np.float32),
        np.asarray(inputs["W1"], np.float32), np.asarray(inputs["b1"], np.float32),
        np.asarray(inputs["W2"], np.float32), np.asarray(inputs["b2"], np.float32),
        use_bf16=USE_BF16)
    nc = _get_program(USE_BF16)

    partition_name = nc.partition_id_tensor.name if nc.partition_id_tensor else None
    in_names, out_names, out_avals, zero_outs = [], [], [], []
    for alloc in nc.m.functions[0].allocations:
        if not isinstance(alloc, mybir.MemoryLocationSet):
            continue
        name = alloc.memorylocations[0].name
        if alloc.kind == "ExternalInput":
            if name != partition_name:
                in_names.append(name)
        elif alloc.kind == "ExternalOutput":
            shape = tuple(alloc.tensor_shape)
            dtype = mybir.dt.np(alloc.dtype)
            out_names.append(name)
            out_avals.append(jax.core.ShapedArray(shape, dtype))
            zero_outs.append(np.zeros(shape, dtype))
    n_params = len(in_names)
    full_in_names = list(in_names) + list(out_names)
    if partition_name is not None:
        full_in_names.append(partition_name)

    def _body(*args):
        operands = list(args)
        if partition_name is not None:
            operands.append(partition_id_tensor())
        outs = _bass_exec_p.bind(
            *operands, out_avals=tuple(out_avals), in_names=tuple(full_in_names),
            out_names=tuple(out_names), lowering_input_output_aliases=(),
            sim_require_finite=True, sim_require_nnan=True, nc=nc)
        return tuple(outs)

    devices = jax.devices()[:N_CORES]
    mesh = Mesh(np.asarray(devices), ("core",))
    in_specs = (PartitionSpec("core"),) * (n_params + len(out_names))
    out_specs = (PartitionSpec("core"),) * len(out_names)
    fn = jax.jit(shard_map(_body, mesh=mesh, in_specs=in_specs,
                           out_specs=out_specs, check_rep=False))

    xs = x.reshape(N_CORES, B_LOC, D)
    in_map = dict(consts, x=None)
    concat_in = []
    for name in in_names:
        if name == "x":
            concat_in.append(x)
        else:
            concat_in.append(np.concatenate([in_map[name]] * N_CORES, axis=0))
    concat_zeros = [np.zeros((N_CORES * z0.shape[0], *z0.shape[1:]), z0.dtype)
                    for z0 in zero_outs]
    args = [jax.device_put(a) for a in concat_in + concat_zeros]

    def run_n(n):
        t0 = time.perf_counter()
        outs = None
        for _ in range(n):
            outs = fn(*args)
        jax.block_until_ready(outs)
        return time.perf_counter() - t0

    run_n(2)  # warm up
    t_short = min(run_n(2) for _ in range(3))
    t_long = min(run_n(2 + iters) for _ in range(3))
    per_iter = (t_long - t_short) / iters
    return per_iter
